# revision 8
# baseline (speedup 1.0000x reference)
"""BinarizedLinear on 8 Trainium2 NeuronCores.

out = x @ sign(weight).T + bias
  x: (32768, 1024) f32, weight: (1024, 1024) f32, bias: (1024,) f32

Strategy (data-parallel over batch, weight/bias replicated):
  - each core handles a 4096-row shard of x
  - host marshals the shard feature-major as fp16 (xT: [1024, 4096]) --
    halves input HBM traffic vs f32 and removes every on-device cast;
    the binarized +-1 weight is exact in fp8 e4m3, shipped pre-transposed
    ([in, out]) and streamed as the matmul moving operand
  - device: x tiles stationary (fp16), weight moving (fp8), K=1024
    accumulated in PSUM over 8 chunks -> DVE bias-add writes fp16 ->
    256KB contiguous stores; host widens fp16 -> f32 (exactly)
  - "dr" mode: the last 2 of 8 K-chunks are carried as fp8 e4m3 pairs and
    fused into one DoubleRow matmul (2 MACs/cell/cycle), trimming PE
    streaming time ~11%; quantization error budget measured at 1.37e-2
    vs the 2e-2 gate
  - warmup matmuls un-throttle the PE clock (HAM) during DMA bring-up
"""

import os
import sys

import numpy as np

sys.path.insert(0, "/opt/trn_rl_repo")

import ml_dtypes

import concourse.tile as tile
from concourse import bacc, mybir
from concourse.bass_utils import run_bass_kernel_spmd

N_CORES = 8
B_FULL = 32768
I_DIM = 1024
O_DIM = 1024
BS = B_FULL // N_CORES  # 4096 batch rows per core

P = 128                # partitions / contraction tile
IC = I_DIM // P        # 8 contraction chunks
N_OC = 512             # psum free width (one PSUM bank of f32)
OC = O_DIM // N_OC     # 2 output chunks
BBLK = 256             # x dma slab width (batch cols)
NBLK = BS // BBLK      # 16 slabs
B_SUB = 128            # stationary-operand free width (psum partitions)

# "fp16": one fp16 x fp8 pass (x rounded to fp16; weight exact).
# "dr":   last 2 K-chunks as one fp8 DoubleRow matmul (faster, more error).
MODE = os.environ.get("BINLIN_MODE", "fp16")

F32 = mybir.dt.float32
FP16 = mybir.dt.float16
FP8 = mybir.dt.float8e4

_cache = {}


def _build_program(mode: str):
    nc = bacc.Bacc("TRN2", target_bir_lowering=False, debug=False,
                   num_devices=N_CORES)

    dr = mode == "dr"
    # K-chunks 0..n_ic16-1 ride fp16; chunks n_ic16..7 ride the DR pair.
    n_ic16 = IC - 2 if dr else IC

    xt = nc.dram_tensor("xt", [I_DIM if not dr else n_ic16 * P, BS], FP16,
                        kind="ExternalInput").ap()
    wt = nc.dram_tensor("wt", [I_DIM, O_DIM], FP8, kind="ExternalInput").ap()
    bias_d = nc.dram_tensor("bias_d", [1, O_DIM], F32,
                            kind="ExternalInput").ap()
    if dr:
        # pairs: xdr[p, j, b] = x[b, 768 + j*128 + p] in e4m3
        xdr = nc.dram_tensor("xdr", [P, 2 * BS], FP8,
                             kind="ExternalInput").ap()
        wdr = nc.dram_tensor("wdr", [P, 2 * O_DIM], FP8,
                             kind="ExternalInput").ap()
    out = nc.dram_tensor("out", [BS, O_DIM], FP16, kind="ExternalOutput").ap()

    with tile.TileContext(nc) as tc:
        with (
            tc.tile_pool(name="consts", bufs=1) as consts,
            tc.tile_pool(name="xb", bufs=NBLK * IC) as xb_pool,
            tc.tile_pool(name="ot", bufs=4) as ot_pool,
            tc.tile_pool(name="ps", bufs=6, space="PSUM") as ps_pool,
        ):
            # PE warmup: data-independent matmuls on scratch SBUF keep the
            # PE busy through DMA bring-up so HAM un-throttles to 2.4 GHz
            # before the first real matmul (results never read).
            warm_sc = consts.tile([P, N_OC], FP16)
            nc.vector.memset(warm_sc[:], 0.0)
            ps_w = ps_pool.tile([P, N_OC], F32, tag="warm", bufs=1)
            for _ in range(6):
                nc.tensor.matmul(ps_w[:], warm_sc[:, :B_SUB], warm_sc[:],
                                 start=True, stop=True, skip_group_check=True)

            # Replicated constants on the scalar-engine HWDGE queue so they
            # don't delay the x stream on sync.
            bias_sb = consts.tile([P, O_DIM], F32)
            nc.scalar.dma_start(bias_sb[:],
                                bias_d[0, :].partition_broadcast(P))
            wt_sb = consts.tile([P, IC, O_DIM], FP8)
            nc.scalar.dma_start(
                wt_sb[:], wt[:, :].rearrange("(ic p) o -> p ic o", p=P))
            if dr:
                wdr_sb = consts.tile([P, 2, O_DIM], FP8)
                nc.scalar.dma_start(wdr_sb[:], wdr[:, :])

            # Whole x shard is SBUF-resident (64KB/partition); emit every
            # load upfront on the sync queue -- Tile back-pressures via the
            # pool and consumers wait on per-tile semaphores.
            xs = {}
            xd = {}
            for blk in range(NBLK):
                b0 = blk * BBLK
                t = xb_pool.tile([P, n_ic16, BBLK], FP16, tag=f"xs_{blk}",
                                 bufs=1)
                nc.sync.dma_start(
                    t[:], xt[:, b0:b0 + BBLK].rearrange(
                        "(ic p) b -> p ic b", p=P))
                xs[blk] = t
                if dr:
                    td = xb_pool.tile([P, 2, BBLK], FP8, tag=f"xdr_{blk}",
                                      bufs=1)
                    nc.sync.dma_start(
                        td[:], xdr[:, :].rearrange("p (j b) -> p j b", j=2)
                        [:, :, b0:b0 + BBLK])
                    xd[blk] = td

            n_mm = n_ic16 + (1 if dr else 0)
            sub_per_blk = BBLK // B_SUB
            for su in range(BS // B_SUB):
                blk, c0 = su // sub_per_blk, (su % sub_per_blk) * B_SUB
                r0 = su * B_SUB
                last = su == BS // B_SUB - 1
                ot = ot_pool.tile([P, O_DIM], FP16, tag="ot")
                for oc in range(OC):
                    ps = ps_pool.tile([P, N_OC], F32, tag="ps")
                    for ic in range(n_ic16):
                        nc.tensor.matmul(
                            ps[:],
                            xs[blk][:, ic, c0:c0 + B_SUB],
                            wt_sb[:, ic, oc * N_OC:(oc + 1) * N_OC],
                            start=(ic == 0),
                            stop=(not dr and ic == n_ic16 - 1),
                        )
                    if dr:
                        nc.tensor.matmul(
                            ps[:],
                            xd[blk][:, :, c0:c0 + B_SUB],
                            wdr_sb[:, :, oc * N_OC:(oc + 1) * N_OC],
                            start=False, stop=True,
                            perf_mode=mybir.MatmulPerfMode.DoubleRow,
                        )
                    nc.vector.tensor_add(
                        ot[:, oc * N_OC:(oc + 1) * N_OC], ps[:],
                        bias_sb[:, oc * N_OC:(oc + 1) * N_OC])
                    if last:
                        # tail: ship each half as soon as it's ready
                        nc.scalar.dma_start(
                            out[r0:r0 + B_SUB, oc * N_OC:(oc + 1) * N_OC],
                            ot[:, oc * N_OC:(oc + 1) * N_OC])
                if not last:
                    # 256KB fully-contiguous store of 128 output rows.
                    nc.scalar.dma_start(out[r0:r0 + B_SUB, :], ot[:])

    nc.compile()
    return nc


def _get_program(mode: str):
    if mode not in _cache:
        _cache[mode] = _build_program(mode)
    return _cache[mode]


def _binarize(weight: np.ndarray) -> np.ndarray:
    s = np.sign(weight)
    s[s == 0] = 1.0
    return s


def kernel_impl(x, weight, bias, mode=MODE, trace=False, tmpdir=None):
    dr = mode == "dr"
    n_ic16 = IC - 2 if dr else IC
    i16 = n_ic16 * P

    s = _binarize(np.asarray(weight, np.float32))
    wt = np.ascontiguousarray(s.T).astype(ml_dtypes.float8_e4m3)
    bias_d = np.ascontiguousarray(np.asarray(bias, np.float32)[None, :])
    x = np.asarray(x, np.float32)
    xT = x.T  # [I, B] view

    if dr:
        # wdr[p, j, o] = sign_w[o, i16 + j*128 + p]
        wdr = np.ascontiguousarray(
            s.T[i16:].reshape(2, P, O_DIM).transpose(1, 0, 2).reshape(
                P, 2 * O_DIM)).astype(ml_dtypes.float8_e4m3)

    in_maps = []
    for c in range(N_CORES):
        sh = xT[:, c * BS:(c + 1) * BS]  # [I, BS]
        m = {"wt": wt, "bias_d": bias_d,
             "xt": np.ascontiguousarray(sh[:i16]).astype(np.float16)}
        if dr:
            m["xdr"] = np.ascontiguousarray(
                sh[i16:].reshape(2, P, BS).transpose(1, 0, 2).reshape(
                    P, 2 * BS)).astype(ml_dtypes.float8_e4m3)
            m["wdr"] = wdr
        in_maps.append(m)

    nc = _get_program(mode)
    try:
        res = run_bass_kernel_spmd(nc, in_maps, list(range(N_CORES)),
                                   trace=trace, tmpdir=tmpdir)
    except Exception:
        # transient runtime hiccups (e.g. first dispatch after long idle)
        res = run_bass_kernel_spmd(nc, in_maps, list(range(N_CORES)),
                                   trace=trace, tmpdir=tmpdir)
    out = np.concatenate(
        [res.results[c]["out"].astype(np.float32) for c in range(N_CORES)],
        axis=0)
    return out, res


def kernel(x, weight, bias):
    out, _ = kernel_impl(x, weight, bias)
    return out


# revision 14
# speedup vs baseline: 1.3596x; 1.3596x over previous
"""BinarizedLinear on 8 Trainium2 NeuronCores.

out = x @ sign(weight).T + bias
  x: (32768, 1024) f32, weight: (1024, 1024) f32, bias: (1024,) f32

Strategy (data-parallel over batch, weight/bias replicated):
  - each core handles a 4096-row shard of x
  - host marshals the shard feature-major as fp16 (xT: [1024, 4096]) --
    halves input HBM traffic vs f32 and removes every on-device cast;
    the binarized +-1 weight is exact in fp8 e4m3, shipped pre-transposed
    ([in, out]) and streamed as the matmul moving operand
  - device: x tiles stationary (fp16), weight moving (fp8), K=1024
    accumulated in PSUM over 8 chunks -> DVE bias-add writes fp16 ->
    256KB contiguous stores; host widens fp16 -> f32 (exactly)
  - "dr" mode: the last 2 of 8 K-chunks are carried as fp8 e4m3 pairs and
    fused into one DoubleRow matmul (2 MACs/cell/cycle), trimming PE
    streaming time ~11%; quantization error budget measured at 1.37e-2
    vs the 2e-2 gate
  - warmup matmuls un-throttle the PE clock (HAM) during DMA bring-up
"""

import os
import sys

import numpy as np

sys.path.insert(0, "/opt/trn_rl_repo")

import ml_dtypes

import concourse.tile as tile
from concourse import bacc, mybir
from concourse.bass_utils import run_bass_kernel_spmd

N_CORES = 8
B_FULL = 32768
I_DIM = 1024
O_DIM = 1024
BS = B_FULL // N_CORES  # 4096 batch rows per core

P = 128                # partitions / contraction tile
IC = I_DIM // P        # 8 contraction chunks
N_OC = 512             # psum free width (one PSUM bank of f32)
OC = O_DIM // N_OC     # 2 output chunks
BBLK = 512             # x dma slab width (batch cols)
NBLK = BS // BBLK      # 8 slabs
B_SUB = 128            # stationary-operand free width (psum partitions)

# "fp16": one fp16 x fp8 pass (x rounded to fp16; weight exact).
# "dr":   last 2 K-chunks as one fp8 DoubleRow matmul (faster, more error).
MODE = os.environ.get("BINLIN_MODE", "fp16")

F32 = mybir.dt.float32
FP16 = mybir.dt.float16
FP8 = mybir.dt.float8e4

_cache = {}


def _build_program(mode: str):
    nc = bacc.Bacc("TRN2", target_bir_lowering=False, debug=False,
                   num_devices=N_CORES)

    dr = mode == "dr"
    # K-chunks 0..n_ic16-1 ride fp16; chunks n_ic16..7 ride the DR pair.
    n_ic16 = IC - 2 if dr else IC

    xt = nc.dram_tensor("xt", [I_DIM if not dr else n_ic16 * P, BS], FP16,
                        kind="ExternalInput").ap()
    wt = nc.dram_tensor("wt", [I_DIM, O_DIM], FP8, kind="ExternalInput").ap()
    bias_d = nc.dram_tensor("bias_d", [P, O_DIM], F32,
                            kind="ExternalInput").ap()
    if dr:
        # pairs: xdr[p, j, b] = x[b, 768 + j*128 + p] in e4m3
        xdr = nc.dram_tensor("xdr", [P, 2 * BS], FP8,
                             kind="ExternalInput").ap()
        wdr = nc.dram_tensor("wdr", [P, 2 * O_DIM], FP8,
                             kind="ExternalInput").ap()
    out = nc.dram_tensor("out", [BS, O_DIM], FP16, kind="ExternalOutput").ap()

    with tile.TileContext(nc) as tc:
        with (
            tc.tile_pool(name="consts", bufs=1) as consts,
            tc.tile_pool(name="xb", bufs=NBLK * IC) as xb_pool,
            tc.tile_pool(name="ot", bufs=4) as ot_pool,
            tc.tile_pool(name="ps", bufs=6, space="PSUM") as ps_pool,
        ):
            # PE warmup: data-independent matmuls on scratch SBUF keep the
            # PE busy through DMA bring-up so HAM un-throttles to 2.4 GHz
            # before the first real matmul (results never read).
            warm_sc = consts.tile([P, N_OC], FP16)
            nc.gpsimd.memset(warm_sc[:], 0.0)
            ps_w = ps_pool.tile([P, N_OC], F32, tag="warm", bufs=1)
            for _ in range(6):
                nc.tensor.matmul(ps_w[:], warm_sc[:, :B_SUB], warm_sc[:],
                                 start=True, stop=True, skip_group_check=True)

            # Replicated constants on the scalar-engine HWDGE queue so they
            # don't delay the x stream on sync. Weight chunks first -- the
            # first matmul group needs all 8; bias isn't read until the
            # first PSUM drain, ~4us later.
            wt_sb = consts.tile([P, IC * O_DIM], FP8)
            for ic in range(IC):
                nc.scalar.dma_start(wt_sb[:, ic * O_DIM:(ic + 1) * O_DIM],
                                    wt[ic * P:(ic + 1) * P, :])
            if dr:
                wdr_sb = consts.tile([P, 2, O_DIM], FP8)
                nc.scalar.dma_start(wdr_sb[:], wdr[:, :])
            bias_sb = consts.tile([P, O_DIM], F32)
            nc.scalar.dma_start(bias_sb[:], bias_d[:, :])

            # Whole x shard is SBUF-resident (64KB/partition); emit every
            # load upfront on the sync queue -- Tile back-pressures via the
            # pool and consumers wait on per-tile semaphores.
            xs = {}
            xd = {}
            for blk in range(NBLK):
                b0 = blk * BBLK
                for ic in range(n_ic16):
                    t = xb_pool.tile([P, BBLK], FP16, tag=f"xs_{blk}_{ic}",
                                     bufs=1)
                    nc.sync.dma_start(t[:], xt[ic * P:(ic + 1) * P,
                                               b0:b0 + BBLK])
                    xs[(blk, ic)] = t
                if dr:
                    td = xb_pool.tile([P, 2, BBLK], FP8, tag=f"xdr_{blk}",
                                      bufs=1)
                    nc.sync.dma_start(
                        td[:], xdr[:, :].rearrange("p (j b) -> p j b", j=2)
                        [:, :, b0:b0 + BBLK])
                    xd[blk] = td

            n_mm = n_ic16 + (1 if dr else 0)
            sub_per_blk = BBLK // B_SUB
            for su in range(BS // B_SUB):
                blk, c0 = su // sub_per_blk, (su % sub_per_blk) * B_SUB
                r0 = su * B_SUB
                last = su == BS // B_SUB - 1
                ot = ot_pool.tile([P, O_DIM], FP16, tag="ot")
                for oc in range(OC):
                    ps = ps_pool.tile([P, N_OC], F32, tag="ps", bufs=7)
                    for ic in range(n_ic16):
                        nc.tensor.matmul(
                            ps[:],
                            xs[(blk, ic)][:, c0:c0 + B_SUB],
                            wt_sb[:, ic * O_DIM + oc * N_OC:
                                  ic * O_DIM + oc * N_OC + N_OC],
                            start=(ic == 0),
                            stop=(not dr and ic == n_ic16 - 1),
                        )
                    if dr:
                        nc.tensor.matmul(
                            ps[:],
                            xd[blk][:, :, c0:c0 + B_SUB],
                            wdr_sb[:, :, oc * N_OC:(oc + 1) * N_OC],
                            start=False, stop=True,
                            perf_mode=mybir.MatmulPerfMode.DoubleRow,
                        )
                    nc.vector.tensor_add(
                        ot[:, oc * N_OC:(oc + 1) * N_OC], ps[:],
                        bias_sb[:, oc * N_OC:(oc + 1) * N_OC])
                    if last:
                        # tail: ship each half as soon as it's ready
                        nc.scalar.dma_start(
                            out[r0:r0 + B_SUB, oc * N_OC:(oc + 1) * N_OC],
                            ot[:, oc * N_OC:(oc + 1) * N_OC])
                if not last:
                    # 256KB fully-contiguous store of 128 output rows.
                    nc.scalar.dma_start(out[r0:r0 + B_SUB, :], ot[:])

    nc.compile()
    return nc


def _get_program(mode: str):
    if mode not in _cache:
        _cache[mode] = _build_program(mode)
    return _cache[mode]


def _binarize(weight: np.ndarray) -> np.ndarray:
    s = np.sign(weight)
    s[s == 0] = 1.0
    return s


def kernel_impl(x, weight, bias, mode=MODE, trace=False, tmpdir=None):
    dr = mode == "dr"
    n_ic16 = IC - 2 if dr else IC
    i16 = n_ic16 * P

    s = _binarize(np.asarray(weight, np.float32))
    wt = np.ascontiguousarray(s.T).astype(ml_dtypes.float8_e4m3)
    bias_d = np.ascontiguousarray(
        np.broadcast_to(np.asarray(bias, np.float32)[None, :], (P, O_DIM)))
    x = np.asarray(x, np.float32)
    xT = x.T  # [I, B] view

    if dr:
        # wdr[p, j, o] = sign_w[o, i16 + j*128 + p]
        wdr = np.ascontiguousarray(
            s.T[i16:].reshape(2, P, O_DIM).transpose(1, 0, 2).reshape(
                P, 2 * O_DIM)).astype(ml_dtypes.float8_e4m3)

    in_maps = []
    for c in range(N_CORES):
        sh = xT[:, c * BS:(c + 1) * BS]  # [I, BS]
        m = {"wt": wt, "bias_d": bias_d,
             "xt": np.ascontiguousarray(sh[:i16]).astype(np.float16)}
        if dr:
            m["xdr"] = np.ascontiguousarray(
                sh[i16:].reshape(2, P, BS).transpose(1, 0, 2).reshape(
                    P, 2 * BS)).astype(ml_dtypes.float8_e4m3)
            m["wdr"] = wdr
        in_maps.append(m)

    nc = _get_program(mode)
    try:
        res = run_bass_kernel_spmd(nc, in_maps, list(range(N_CORES)),
                                   trace=trace, tmpdir=tmpdir)
    except Exception:
        # transient runtime hiccups (e.g. first dispatch after long idle)
        res = run_bass_kernel_spmd(nc, in_maps, list(range(N_CORES)),
                                   trace=trace, tmpdir=tmpdir)
    out = np.concatenate(
        [res.results[c]["out"].astype(np.float32) for c in range(N_CORES)],
        axis=0)
    return out, res


def kernel(x, weight, bias):
    out, _ = kernel_impl(x, weight, bias)
    return out
